# revision 45
# baseline (speedup 1.0000x reference)
"""DeepAR (2-layer LSTM encoder + LSTM-cell decoder) Trainium2 Bass kernel.

Sharding: pure data parallel, batch 1024 -> 128 per core across 8 cores
(batch 128 == SBUF partition width).

Per-core design (fp8-DoubleRow encoder, engine-balanced; measured
1.56ms vs the 1.83ms per-chunk-ACT baseline):
  - gates in [128 batch, 2048 gate] layout, reordered to [g, i, f, o];
    per cell: g in a 1-bank PSUM tile (tanh), [i,f] merged in a 2-bank
    tile (one sigmoid ACT covers both: ACT ~(172+FD)/1.2GHz, so fewer,
    larger ops), [o] in a 1-bank tile. Chunks are EMITTED [i,f,g,o]
    (CH_ORD) so sigmoid([i,f]) - the head of the c-update chain - is
    ready after only 4 DR matmuls, with tanh(g) overlapping behind it.
  - all DVE elementwise work in bf16 (2x_1p mode on packed 16-bit).
  - encoder recurrent matmuls (h0@W_hh0, h0@W_ih1, h1@W_hh1) in fp8e4
    perf_mode=DoubleRow (~215ns per K=256,N=512 mm at full clock).
    Weights and h pre-scaled by 16 each; x-side weights/biases scaled
    by 256 in bf16; gate ACT ops apply 1/256.
  - the cell tail is computed in TRANSPOSED form: sigmoid(o).T via the
    otherwise-idle DMA xbar (~2us of slack), c.T via 4 PE transposes,
    tanh on the transposed PSUM, then ONE scalar_tensor_tensor
    (tanh(cT)*16)*soT fuses h-mul + fp8 scale + cast. h never exists in
    [batch,h] layout; the recurrence loop is add -> 4 transposes ->
    tanh -> stt -> next DR matmuls.
  - cell1 is software-pipelined two-deep: iteration t runs L1's
    matmuls for step t-1 (all four K=1 bias mms emitted TOGETHER right
    before wi1 - by then every PSUM ring is released, so they issue
    back-to-back into concurrent 32-row groups instead of staggered
    solo slots; wh1 LAST - its h1T8 lands mid-iteration), cell1(t-1)'s
    ACTs+c-update (executing across the period boundary), and
    cell1(t-2)'s transposed tail. Every emitted op is near-ready when
    its engine reaches it; critically the PE never idles (a PE idle
    trips the DVFS throttle and halves the clock for the next ~3us of
    matmuls, which is how a 1.2us bubble used to cost ~3us/step).
  - engine OOO is only a 4-deep wait window per engine: never emit >3
    consecutive not-yet-ready ops ahead of ready work or the queue
    head-of-line blocks.
  - PSUM budget exactly 8 banks: gt ring 2 (g chunks, stable 2-allocs/
    iteration phase) + L0 [i,f] 2 + L0-o/c0T staging 1 (tag gos) +
    L1 [i,f] 2 + L1-o/c1T staging 1 (tag l1o). Staging tiles reuse the
    o-gate bank after its sigmoid ACT read; ring slot phase must be
    STABLE across iterations (an odd alloc count per iteration on a
    2-ring couples the chain to the slow consumer every other step).
  - decoder kept in bf16 [batch,h] form (fp8 hd fails the 2e-2 gate;
    the DMA-xbar so.T latency sits on its single-cell recurrence, so
    its tail stays tanh -> h-mul -> 4 PE transposes -> SBUF copy);
    context injected per step via identity matmul; mu/sigma heads are
    DVE dot-products with accumulate.
  - NOTE: scaling the transpose identity to fold in the x16 does NOT
    work - the PE transpose path ignores the identity's values.
"""
import numpy as np
import ml_dtypes

import concourse.bass as bass
import concourse.mybir as mybir
import concourse.tile as tile
from concourse import bacc
from concourse.bass_utils import run_bass_kernel_spmd
from concourse.masks import make_identity

F32 = mybir.dt.float32
BF16 = mybir.dt.bfloat16
FP8 = mybir.dt.float8e4
AF = mybir.ActivationFunctionType
ALU = mybir.AluOpType
DR = mybir.MatmulPerfMode.DoubleRow

B, T_ENC, H_DEC = 1024, 168, 24
ENC_IN, DEC_IN, HID = 32, 16, 512
G = 4 * HID  # 2048
NCORES = 8
BL = B // NCORES  # 128 batch per core
XCHUNK = 28  # encoder-input steps per DMA chunk

WSCALE = 16.0  # fp8 weight pre-scale
HSCALE = 16.0  # fp8 h pre-scale
GSCALE = 1.0 / (WSCALE * HSCALE)  # ACT de-scale on gate reads

# gate reorder: torch order [i, f, g, o] -> [g, i, f, o]
_PERM = np.concatenate([np.arange(1024, 1536), np.arange(0, 512),
                        np.arange(512, 1024), np.arange(1536, 2048)])


def _bf16(x):
    return np.ascontiguousarray(x.astype(ml_dtypes.bfloat16))


def _fp8(x):
    return np.ascontiguousarray(
        np.clip(x, -224.0, 224.0).astype(ml_dtypes.float8_e4m3))


def _f32(x):
    return np.ascontiguousarray(x.astype(np.float32))


def _wT_kxn(W, conv=_bf16, scale=1.0):
    """[4H, D] gate-major weight -> reordered W.T as [128, D//128, 4H]."""
    Wt = W[_PERM].T * scale  # [D, 2048]
    D = Wt.shape[0]
    return conv(Wt.reshape(D // 128, 128, G).transpose(1, 0, 2))


def build_kernel(T=T_ENC, HD=H_DEC):
    nc = bacc.Bacc("TRN2", target_bir_lowering=False, debug=False,
                   num_devices=NCORES)

    def din(name, shape, dt):
        return nc.dram_tensor(name, shape, dt, kind="ExternalInput").ap()

    x_d = din("x", [ENC_IN, T, BL], BF16)  # enc features (no ones row)
    w0_d = din("w0", [128, G], BF16)   # W_ih0T*256 replicated at 4x32-row bands
    b04_d = din("b04", [128, 640], BF16)  # b0*256 chunks at parts 0/32/64/96
    wh0_d = din("wh0", [128, 4, G], FP8)              # *16
    wi1_d = din("wi1", [128, 4, G], FP8)              # *16
    wh1_d = din("wh1", [128, 4, G], FP8)              # *16
    wctx_d = din("wctx", [128, 4, G], BF16)
    whd_d = din("whd", [128, 4, G], BF16)
    be_d = din("be", [33, G + 128], BF16)  # row32: bd|ones (decoder)
    # b1*256 by chunk at partitions 0/32/64/96: cols 0:128 ones, 128:640 bias
    b14_d = din("b14", [128, 640], BF16)
    covy_d = din("covy", [128, HD, BL], BF16)  # dec cov+y at parts 0/32/64/96
    wcy_d = din("wcy", [128, G], BF16)         # replicated at parts 0/32/64/96
    # head weights broadcast across partitions + per-partition biases:
    # cols 0:512 W_mu, 512:1024 W_sig, 1024 b_mu, 1025 b_sig
    wms_d = din("wms", [128, 2 * HID + 2], F32)

    mu_d = nc.dram_tensor("mu", [BL, HD], F32, kind="ExternalOutput").ap()
    sg_d = nc.dram_tensor("sg", [BL, HD], F32, kind="ExternalOutput").ap()

    with tile.TileContext(nc) as tc:
        _emit(tc, T, HD, x_d, w0_d, b04_d, wh0_d, wi1_d, wh1_d, wctx_d,
              whd_d, be_d, b14_d, covy_d, wcy_d, wms_d, mu_d, sg_d)
    nc.compile()
    return nc


def _emit(tc, T, HD, x_d, w0_d, b04_d, wh0_d, wi1_d, wh1_d, wctx_d, whd_d,
          be_d, b14_d, covy_d, wcy_d, wms_d, mu_d, sg_d):
    nc = tc.nc
    mm = nc.tensor.matmul
    NS = 4  # gate chunks [g, i, f, o]
    CH_ORD = (1, 2, 0, 3)  # emission order [i, f, g, o]: sig([i,f]) first

    with (
        tc.tile_pool(name="const", bufs=1) as cp,
        tc.tile_pool(name="xp", bufs=2) as xp,
        tc.tile_pool(name="sig", bufs=3) as sigp,
        tc.tile_pool(name="small", bufs=3) as smp,
        tc.tile_pool(name="hp", bufs=3) as hp,
        tc.tile_pool(name="htp", bufs=3) as htp,
        tc.tile_pool(name="ht8p", bufs=4) as ht8p,
        # PSUM: 8 banks exactly. tag gt (2 bufs x 1 bank) g chunks;
        # gif (1 x 2 banks) L0 [i,f]; gos (1 x 1) L0 [o] + c0T staging;
        # l1if (1 x 2) L1/dec [i,f]; l1o (1 x 1) L1/dec [o] + c1T staging.
        tc.tile_pool(name="psum", bufs=1, space="PSUM") as pp,
    ):
        # ---- persistent tiles / weight loads ----
        def load(name, dram, shape, dt):
            t = cp.tile(shape, dt, tag=name, name=name)
            nc.sync.dma_start(t[:], dram[:])
            return t

        w0 = load("w0", w0_d, [128, G], BF16)
        b04 = load("b04", b04_d, [128, 640], BF16)
        wh0 = load("wh0", wh0_d, [128, 4, G], FP8)
        be = load("be", be_d, [33, G + 128], BF16)
        b14 = load("b14", b14_d, [128, 640], BF16)
        wi1 = load("wi1", wi1_d, [128, 4, G], FP8)
        wh1 = load("wh1", wh1_d, [128, 4, G], FP8)

        ident = cp.tile([128, 128], BF16, tag="ident")
        make_identity(nc, ident[:])

        ones32_r = be[32:33, G:G + 128]
        bd_r = be[32:33, 0:G]

        c0 = cp.tile([128, HID], BF16, tag="c0")
        c1 = cp.tile([128, HID], BF16, tag="c1")
        cd = cp.tile([128, HID], BF16, tag="cd")
        mu_b = cp.tile([128, HD], F32, tag="mu_b")
        sp_b = cp.tile([128, HD], F32, tag="sp_b")
        sg_b = cp.tile([128, HD], F32, tag="sg_b")

        def psum_cell(kind):
            """Allocate one cell's gate PSUM tiles -> (dsts, gt, gif, go).

            All cells split as: g 1-bank (tanh), [i,f] 2-bank, [o] 1-bank.
            L0 and L1 use distinct [i,f]/[o] tags so each ring has a
            stable slot per step (no phase alternation); L0's o bank
            doubles as the transpose staging bank (tag gos).
            """
            gt = pp.tile([128, 512], F32, tag="gt", bufs=2, name="gt")
            if kind == "l0":
                gif = pp.tile([128, 1024], F32, tag="gif", bufs=1, name="gif")
                go = pp.tile([128, 512], F32, tag="gos", bufs=1, name="go")
            else:
                gif = pp.tile([128, 1024], F32, tag="l1if", bufs=1, name="gif1")
                go = pp.tile([128, 512], F32, tag="l1o", bufs=1, name="go1")
            dsts = [gt[:], gif[:, 0:512], gif[:, 512:1024], go[:]]
            return dsts, gt, gif, go

        def cell_acts(gt, gif, go, scale):
            """ACT ops for one cell, in chunk-arrival order: the [i,f]
            chunks are emitted first on the PE (see CH_ORD), so
            sigmoid([i,f]) leads the ACT queue and tanh(g) overlaps
            behind it."""
            sif = sigp.tile([128, 1024], BF16, tag="sif")
            nc.scalar.activation(sif[:], gif[:], AF.Sigmoid, scale=scale)
            tg = smp.tile([128, HID], BF16, tag="tg")
            nc.scalar.activation(tg[:], gt[:], AF.Tanh, scale=scale)
            so = sigp.tile([128, HID], BF16, tag="so")
            nc.scalar.activation(so[:], go[:], AF.Sigmoid, scale=scale)
            return tg, sif[:, 0:512], sif[:, 512:1024], so

        def cell_core(tg, si, sf, c, first):
            """DVE c-update (bf16 2x) + tanh(c') on ACT -> tcn tile."""
            if first:
                nc.vector.tensor_mul(c[:], si, tg[:])
            else:
                m1 = smp.tile([128, HID], BF16, tag="m1")
                nc.vector.tensor_mul(m1[:], si, tg[:])
                m2 = smp.tile([128, HID], BF16, tag="m2")
                nc.vector.tensor_mul(m2[:], sf, c[:])
                nc.vector.tensor_add(c[:], m1[:], m2[:])
            tcn = smp.tile([128, HID], BF16, tag="tc")
            nc.scalar.activation(tcn[:], c[:], AF.Tanh)
            return tcn

        def cell_dve(tg, si, sf, so, c, first, h_tag):
            """Full cell tail: h = so * tanh(c') (used by L1/decoder)."""
            tcn = cell_core(tg, si, sf, c, first)
            h = hp.tile([128, HID], BF16, tag=h_tag)
            nc.vector.tensor_mul(h[:], so[:], tcn[:])
            return h

        def pe_transp(h, tag):
            """[128,512] bf16 SBUF -> [128,4,128] bf16 PSUM via 4 PE
            transposes into a staging tile sharing a dead gate bank
            (tag gos = o-chunk bank, tag gif = i/f banks)."""
            ht = pp.tile([128, 4, 128], BF16, tag=tag, bufs=1, name="hT")
            for k in range(4):
                nc.tensor.transpose(ht[:, k, :], h[:, k * 128:(k + 1) * 128],
                                    ident[:])
            return ht

        # ================= encoder =================
        # L1 runs one step behind L0: while L0(t)'s elementwise chain runs
        # on ACT/DVE, the PE stays busy on L1(t-1)'s matmuls.
        h0T8_hist = {}

        x_cur = None
        x_nxt = None

        def load_xchunk(t0):
            """x replicated at partitions 0 and 64 for 2-way row tiling."""
            nxc = min(XCHUNK, T - t0)
            xt = xp.tile([128, XCHUNK, BL], BF16, tag="x")
            for b in range(4):
                nc.sync.dma_start(xt[32 * b:32 * b + ENC_IN, :nxc, :],
                                  x_d[:, t0:t0 + nxc, :])
            return xt

        def l1_bias(d1):
            """K=1 bias matmuls, 4-wide concurrent row groups."""
            for n in CH_ORD:
                bp = 32 * n
                mm(d1[n], b14[bp:bp + 1, 0:128], b14[bp:bp + 1, 128:640],
                   tile_position=(bp, 0), start=True, stop=False)

        def l1_wi1(d1, tp, hp8):
            """wi1 matmuls for layer-1 step tp (all four chunks)."""
            for n in CH_ORD:
                s = slice(n * 512, (n + 1) * 512)
                mm(d1[n], hp8[:, 0:2, :], wi1[:, 0:2, s],
                   perf_mode=DR, start=False, stop=False)
                mm(d1[n], hp8[:, 2:4, :], wi1[:, 2:4, s],
                   perf_mode=DR, start=False, stop=(tp == 0))

        def l1_wh1(d1, h1T8, chunks):
            """wh1 matmuls for the given chunk indices (region stops)."""
            for n in chunks:
                s = slice(n * 512, (n + 1) * 512)
                mm(d1[n], h1T8[:, 0:2, :], wh1[:, 0:2, s],
                   perf_mode=DR, start=False, stop=False)
                mm(d1[n], h1T8[:, 2:4, :], wh1[:, 2:4, s],
                   perf_mode=DR, start=False, stop=True)

        def mchain(tg, si, sf, c, first):
            """DVE c-update (bf16 2x): c' = sf*c + si*tg (in place)."""
            if first:
                nc.vector.tensor_mul(c[:], si, tg[:])
            else:
                m1 = smp.tile([128, HID], BF16, tag="m1")
                nc.vector.tensor_mul(m1[:], si, tg[:])
                m2 = smp.tile([128, HID], BF16, tag="m2")
                nc.vector.tensor_mul(m2[:], sf, c[:])
                nc.vector.tensor_add(c[:], m1[:], m2[:])

        def cell_tail(so, c, ring, so_tag, tc_tag, so_dma=True):
            """Transposed cell tail. In the encoder sigmoid(o).T rides the
            otherwise-idle DMA xbar straight into SBUF (it has ~2us of
            slack before the stt consumes it, saving 4 PE transposes + a
            DVE copy per cell); in the decoder the xbar's ~1.5us latency
            would sit on the recurrence loop, so it stays on the PE.
            c.T takes 4 PE transposes (chain-critical), with tanh
            computed on the transposed PSUM. The h-mul itself is fused
            into the caller's stt/mul, so nothing on the critical path
            waits for a [b,h]-layout h that no one needs."""
            soT_sb = hp.tile([128, 4, 128], BF16, tag=so_tag)
            if so_dma:
                nc.sync.dma_start_transpose(soT_sb[:], so[:])
            else:
                soT = pe_transp(so, ring)
                nc.vector.tensor_copy(soT_sb[:], soT[:])
            cT = pe_transp(c, ring)
            tcnT = smp.tile([128, 4, 128], BF16, tag=tc_tag)
            nc.scalar.activation(tcnT[:], cT[:], AF.Tanh)
            return soT_sb, tcnT

        # cell1 is software-pipelined two-deep: iteration t runs
        #   - L1 matmuls for step t-1 (bias+wi1+wh1),
        #   - cell1(t-1)'s ACTs + c-update (emitted last, execute across
        #     the period boundary),
        #   - cell1(t-2)'s transposed tail -> h1T8(t-2), consumed by
        #     wh1(t-1) in this same iteration.
        # This keeps every emitted op near-ready when its engine reaches
        # it, so the h1 recurrence loop never sets the period.
        h1T8_hist = {}
        pend_so1 = None  # sigmoid(o1) of step t-1 awaiting its transpose

        for t in range(T):
            if t == 0:
                x_cur = load_xchunk(0)
                if T > XCHUNK:
                    x_nxt = load_xchunk(XCHUNK)
            elif t % XCHUNK == 0:
                x_cur = x_nxt
                if t + XCHUNK < T:
                    x_nxt = load_xchunk(t + XCHUNK)
            ti = t % XCHUNK

            # ---- layer 0 step t ----
            # Chunk emission order [i, f, g, o] (CH_ORD): sigmoid([i,f])
            # — the head of the c-update chain — becomes ready after only
            # 4 DR matmuls, with tanh(g) overlapping behind it on ACT.
            # The in-mms pair by PE row group: {i(64)||f(0)}, {g(0)||o(64)};
            # with the L1 bias they also cover the stt0(t-1) chain tail so
            # the PE never idles at the step boundary (idle trips the DVFS
            # throttle: half clock for the next ~3us of matmuls).
            d0, gt0, gif0, go0 = psum_cell("l0")
            hp8 = h0T8_hist.get(t - 1)
            # L0's PSUM rings all release before the boundary, so the K=1
            # bias mms and the K=32 feature mms each issue as one 4-wide
            # concurrent slot (b0 moved off the x ones-row to allow
            # 32-row tiles).
            for n in CH_ORD:
                bp = 32 * n
                mm(d0[n], b04[bp:bp + 1, 0:128], b04[bp:bp + 1, 128:640],
                   tile_position=(bp, 0), start=True, stop=False)
            for n in CH_ORD:
                rb = 32 * n
                mm(d0[n], x_cur[rb:rb + ENC_IN, ti, :],
                   w0[rb:rb + ENC_IN, n * 512:(n + 1) * 512],
                   tile_position=(rb, 0), start=False, stop=(t == 0))
            if t > 0:
                for n in CH_ORD:
                    s = slice(n * 512, (n + 1) * 512)
                    mm(d0[n], hp8[:, 0:2, :], wh0[:, 0:2, s],
                       perf_mode=DR, start=False, stop=False)
                    mm(d0[n], hp8[:, 2:4, :], wh0[:, 2:4, s],
                       perf_mode=DR, start=False, stop=True)
            # cell0 ACTs (engine waits on the mms via semaphores)
            tg0, si0, sf0, so0 = cell_acts(gt0, gif0, go0, GSCALE)

            # cell1(t-2) transposed tail -> h1T8(t-2) (all inputs are from
            # the previous iteration, so these run without stalling)
            if t >= 2:
                so1T_sb, tcn1T = cell_tail(pend_so1, c1, "l1o", "so1T",
                                           "tc1T")
                h1T8 = ht8p.tile([128, 4, 128], FP8, tag="h1T8")
                nc.vector.scalar_tensor_tensor(
                    h1T8[:], tcn1T[:], HSCALE, so1T_sb[:],
                    op0=ALU.mult, op1=ALU.mult)
                h1T8_hist[t - 2] = h1T8
                h1T8_hist.pop(t - 4, None)

            # L1 psum + ALL FOUR bias mms emitted together here: by the
            # time the PE reaches them every ring is released, so they
            # issue back-to-back into 4 concurrent 32-row groups (one
            # ~310ns slot instead of three staggered ones).
            if t >= 1:
                d1, gt1, gif1, go1 = psum_cell("l1")
                l1_bias(d1)
                l1_wi1(d1, t - 1, h0T8_hist[t - 1])

            # cell0 c-update, then its transposed tail -> h0T8(t)
            mchain(tg0, si0, sf0, c0, t == 0)
            so0T_sb, tcn0T = cell_tail(so0, c0, "gos", "so0T", "tc0T")
            h0T8 = ht8p.tile([128, 4, 128], FP8, tag="h0T8")
            nc.vector.scalar_tensor_tensor(
                h0T8[:], tcn0T[:], HSCALE, so0T_sb[:],
                op0=ALU.mult, op1=ALU.mult)
            h0T8_hist[t] = h0T8
            h0T8_hist.pop(t - 2, None)

            # wh1(t-1) last: h1T8(t-2) lands mid-iteration; these also
            # give the PE tail work past the stt0 chain tail.
            if t >= 2:
                l1_wh1(d1, h1T8_hist[t - 2], CH_ORD)

            # cell1(t-1) ACTs + c-update: execute across the period
            # boundary; the tail runs next iteration.
            if t >= 1:
                tg1, si1, sf1, so1 = cell_acts(gt1, gif1, go1, GSCALE)
                mchain(tg1, si1, sf1, c1, t == 1)
                pend_so1 = so1

        # ---- epilogue: cell1(T-2) tail, then the full final L1 step ----
        if T >= 2:
            so1T_sb, tcn1T = cell_tail(pend_so1, c1, "l1o", "so1T", "tc1T")
            h1T8 = ht8p.tile([128, 4, 128], FP8, tag="h1T8")
            nc.vector.scalar_tensor_tensor(
                h1T8[:], tcn1T[:], HSCALE, so1T_sb[:],
                op0=ALU.mult, op1=ALU.mult)
            h1T8_hist[T - 2] = h1T8
        d1, gt1, gif1, go1 = psum_cell("l1")
        l1_bias(d1)
        l1_wi1(d1, T - 1, h0T8_hist[T - 1])
        if T - 1 > 0:
            l1_wh1(d1, h1T8_hist[T - 2], CH_ORD)
        tg1, si1, sf1, so1 = cell_acts(gt1, gif1, go1, GSCALE)
        mchain(tg1, si1, sf1, c1, T == 1)
        # final h1 in transposed bf16 [128,4,128] for the ctx GEMM
        so1T_sb, tcn1T = cell_tail(so1, c1, "l1o", "so1T", "tc1T")
        h1T = htp.tile([128, 4, 128], BF16, tag="h1T")
        nc.vector.tensor_mul(h1T[:], tcn1T[:], so1T_sb[:])

        # ================= decoder (bf16) =================
        wctx = load("wctx", wctx_d, [128, 4, G], BF16)
        whd = load("whd", whd_d, [128, 4, G], BF16)
        covy = load("covy", covy_d, [128, HD, BL], BF16)
        wcy = load("wcy", wcy_d, [128, G], BF16)
        wms = load("wms", wms_d, [128, 2 * HID + 2], F32)
        # one-time: ctx_pre = context @ W_ctx.T + (b_ihd + b_hhd);
        # bias rides K=1 mms off the be ones row.
        cdst, ctg, ctif, cto = psum_cell("l1")
        for n in range(NS):
            s = slice(n * 512, (n + 1) * 512)
            mm(cdst[n], ones32_r, bd_r[:, s], start=True, stop=False)
        for k in range(4):
            for n in range(NS):
                s = slice(n * 512, (n + 1) * 512)
                mm(cdst[n], h1T[:, k, :], wctx[:, k, s],
                   start=False, stop=(k == 3))
        ctxp = cp.tile([128, G], BF16, tag="ctxp")
        nc.scalar.copy(ctxp[:, 0:512], ctg[:])
        nc.scalar.copy(ctxp[:, 512:1536], ctif[:])
        nc.scalar.copy(ctxp[:, 1536:2048], cto[:])

        hdT = None
        for t in range(HD):
            dd, dgt, dif, dgo = psum_cell("l1")
            for n in CH_ORD:
                s = slice(n * 512, (n + 1) * 512)
                mm(dd[n], ident[:], ctxp[:, s], start=True, stop=False)
            for n in CH_ORD:
                s = slice(n * 512, (n + 1) * 512)
                rb = 32 * n
                mm(dd[n], covy[rb:rb + DEC_IN + 1, t, :],
                   wcy[rb:rb + DEC_IN + 1, s], tile_position=(rb, 0),
                   start=False, stop=(t == 0))
            if t > 0:
                for n in CH_ORD:
                    s = slice(n * 512, (n + 1) * 512)
                    for k in range(4):
                        mm(dd[n], hdT[:, k, :], whd[:, k, s],
                           start=False, stop=(k == 3))
            tgd, sid, sfd, sod = cell_acts(dgt, dif, dgo, 1.0)
            hd = cell_dve(tgd, sid, sfd, sod, cd, t == 0, "hd")
            hdT_ps = pe_transp(hd, "gos")
            hdT = htp.tile([128, 4, 128], BF16, tag="hdT")
            nc.vector.tensor_copy(hdT[:], hdT_ps[:])

            # heads: mu/sigma dot-products on DVE, off the critical path
            hsc = smp.tile([128, HID], F32, tag="hsc")
            nc.vector.scalar_tensor_tensor(
                hsc[:], hd[:], 1.0, wms[:, 0:HID],
                op0=ALU.mult, op1=ALU.mult, accum_out=mu_b[:, t:t + 1])
            hsc2 = smp.tile([128, HID], F32, tag="hsc2")
            nc.vector.scalar_tensor_tensor(
                hsc2[:], hd[:], 1.0, wms[:, HID:2 * HID],
                op0=ALU.mult, op1=ALU.mult, accum_out=sp_b[:, t:t + 1])

        # add head biases; sigma = softplus(x) + 1e-6 via ln(exp(x)+1)
        nc.vector.tensor_scalar_add(mu_b[:], mu_b[:],
                                    wms[:, 2 * HID:2 * HID + 1])
        nc.vector.tensor_scalar_add(sp_b[:], sp_b[:],
                                    wms[:, 2 * HID + 1:2 * HID + 2])
        nc.scalar.activation(sp_b[:], sp_b[:], AF.Exp)
        nc.scalar.activation(sg_b[:], sp_b[:], AF.Ln, bias=1.0)
        nc.vector.tensor_scalar_add(sg_b[:], sg_b[:], 1e-6)
        nc.sync.dma_start(mu_d[:], mu_b[:])
        nc.sync.dma_start(sg_d[:], sg_b[:])


def _make_be(bdv):
    be = np.zeros((33, G + 128), np.float32)
    be[32, :G] = bdv
    be[32, G:] = 1.0
    return _bf16(be)


def _make_b14(b1):
    """b1*256 chunks at partitions 0/32/64/96 for 4-wide K=1 row tiling."""
    b = np.zeros((128, 640), np.float32)
    for i in range(4):
        b[32 * i, 0:128] = 1.0
        b[32 * i, 128:640] = b1[i * 512:(i + 1) * 512] / GSCALE
    return _bf16(b)


def _make_wms(W_mu, W_sig, b_mu, b_sig):
    w = np.zeros((128, 2 * HID + 2), np.float32)
    w[:, 0:HID] = W_mu[0][None, :]
    w[:, HID:2 * HID] = W_sig[0][None, :]
    w[:, 2 * HID] = b_mu[0]
    w[:, 2 * HID + 1] = b_sig[0]
    return _f32(w)


def prep_inputs(inputs, T=T_ENC, HD=H_DEC):
    """Full-batch inputs -> list of per-core input maps (host layout prep)."""
    enc = _f32(np.asarray(inputs["enc_inp"]))[:, :T]
    dec = _f32(np.asarray(inputs["dec_inp"]))[:, :HD]
    tgt = _f32(np.asarray(inputs["tgt"]))[:, :HD]

    W_ih0, W_hh0 = np.asarray(inputs["W_ih0"]), np.asarray(inputs["W_hh0"])
    W_ih1, W_hh1 = np.asarray(inputs["W_ih1"]), np.asarray(inputs["W_hh1"])
    W_ihd, W_hhd = np.asarray(inputs["W_ihd"]), np.asarray(inputs["W_hhd"])
    b0 = _f32(np.asarray(inputs["b_ih0"]) + np.asarray(inputs["b_hh0"]))[_PERM]
    b1 = _f32(np.asarray(inputs["b_ih1"]) + np.asarray(inputs["b_hh1"]))[_PERM]
    bdv = _f32(np.asarray(inputs["b_ihd"]) + np.asarray(inputs["b_hhd"]))[_PERM]
    W_mu, b_mu = np.asarray(inputs["W_mu"]), np.asarray(inputs["b_mu"])
    W_sig, b_sig = np.asarray(inputs["W_sig"]), np.asarray(inputs["b_sig"])

    # x-side weights *256 in bf16 (exact power-of-two scale); gate reads
    # apply scale=1/256. w0 replicated at all four 32-row bands for the
    # 4-wide K=32 input matmuls; b0 goes through b04 K=1 mms.
    w0 = W_ih0[_PERM].T / GSCALE  # [32, 2048]
    w02 = np.zeros((128, G), np.float32)
    for i in range(4):
        w02[32 * i:32 * i + ENC_IN] = w0
    wcy1 = np.concatenate(
        [W_ihd[_PERM][:, :DEC_IN].T, W_ihd[_PERM][:, DEC_IN + HID:].T], 0)
    wcy4 = np.zeros((128, G), np.float32)
    for i in range(4):
        wcy4[32 * i:32 * i + DEC_IN + 1] = wcy1
    shared = {
        "w0": _bf16(w02),
        "wh0": _wT_kxn(W_hh0, conv=_fp8, scale=WSCALE),
        "wi1": _wT_kxn(W_ih1, conv=_fp8, scale=WSCALE),
        "wh1": _wT_kxn(W_hh1, conv=_fp8, scale=WSCALE),
        "wctx": _wT_kxn(W_ihd[:, DEC_IN:DEC_IN + HID]),
        "whd": _wT_kxn(W_hhd),
        "be": _make_be(bdv),
        "b14": _make_b14(b1),
        "b04": _make_b14(b0),
        "wcy": _bf16(wcy4),
        "wms": _make_wms(W_mu, W_sig, b_mu, b_sig),
    }

    in_maps = []
    for c in range(NCORES):
        sl = slice(c * BL, (c + 1) * BL)
        xe = _f32(enc[sl].transpose(2, 1, 0))
        cy1 = np.zeros((DEC_IN + 1, HD, BL), np.float32)
        cy1[:DEC_IN] = dec[sl].transpose(2, 1, 0)
        cy1[DEC_IN, 1:] = tgt[sl, :HD - 1].T
        cy = np.zeros((128, HD, BL), np.float32)
        for i in range(4):
            cy[32 * i:32 * i + DEC_IN + 1] = cy1
        m = dict(shared)
        m["x"] = _bf16(xe)
        m["covy"] = _bf16(cy)
        in_maps.append(m)
    return in_maps


_NC_CACHE = {}


def _get_nc(T=T_ENC, HD=H_DEC):
    key = (T, HD)
    if key not in _NC_CACHE:
        _NC_CACHE[key] = build_kernel(T, HD)
    return _NC_CACHE[key]


def run(inputs, T=T_ENC, HD=H_DEC, **kw):
    nc = _get_nc(T, HD)
    in_maps = prep_inputs(inputs, T, HD)
    res = run_bass_kernel_spmd(nc, in_maps, core_ids=list(range(NCORES)), **kw)
    mu = np.concatenate([res.results[c]["mu"] for c in range(NCORES)], 0)
    sg = np.concatenate([res.results[c]["sg"] for c in range(NCORES)], 0)
    return (mu, sg), res


def kernel(**inputs):
    (mu, sg), _ = run(inputs)
    return mu, sg


# revision 46
# speedup vs baseline: 1.0474x; 1.0474x over previous
"""DeepAR (2-layer LSTM encoder + LSTM-cell decoder) Trainium2 Bass kernel.

Sharding: pure data parallel, batch 1024 -> 128 per core across 8 cores
(batch 128 == SBUF partition width).

Per-core design (fp8-DoubleRow encoder, engine-balanced; measured
1.56ms vs the 1.83ms per-chunk-ACT baseline):
  - gates in [128 batch, 2048 gate] layout, reordered to [g, i, f, o];
    per cell: g in a 1-bank PSUM tile (tanh), [i,f] merged in a 2-bank
    tile (one sigmoid ACT covers both: ACT ~(172+FD)/1.2GHz, so fewer,
    larger ops), [o] in a 1-bank tile. Chunks are EMITTED [i,f,g,o]
    (CH_ORD) so sigmoid([i,f]) - the head of the c-update chain - is
    ready after only 4 DR matmuls, with tanh(g) overlapping behind it.
  - all DVE elementwise work in bf16 (2x_1p mode on packed 16-bit).
  - encoder recurrent matmuls (h0@W_hh0, h0@W_ih1, h1@W_hh1) in fp8e4
    perf_mode=DoubleRow (~215ns per K=256,N=512 mm at full clock).
    Weights and h pre-scaled by 16 each; x-side weights/biases scaled
    by 256 in bf16; gate ACT ops apply 1/256.
  - the cell tail is computed in TRANSPOSED form: sigmoid(o).T via the
    otherwise-idle DMA xbar (~2us of slack), c.T via 4 PE transposes,
    tanh on the transposed PSUM, then ONE scalar_tensor_tensor
    (tanh(cT)*16)*soT fuses h-mul + fp8 scale + cast. h never exists in
    [batch,h] layout; the recurrence loop is add -> 4 transposes ->
    tanh -> stt -> next DR matmuls.
  - cell1 is software-pipelined two-deep: iteration t runs L1's
    matmuls for step t-1 (all four K=1 bias mms emitted TOGETHER right
    before wi1 - by then every PSUM ring is released, so they issue
    back-to-back into concurrent 32-row groups instead of staggered
    solo slots; wh1 LAST - its h1T8 lands mid-iteration), cell1(t-1)'s
    ACTs+c-update (executing across the period boundary), and
    cell1(t-2)'s transposed tail. Every emitted op is near-ready when
    its engine reaches it; critically the PE never idles (a PE idle
    trips the DVFS throttle and halves the clock for the next ~3us of
    matmuls, which is how a 1.2us bubble used to cost ~3us/step).
  - engine OOO is only a 4-deep wait window per engine: never emit >3
    consecutive not-yet-ready ops ahead of ready work or the queue
    head-of-line blocks.
  - PSUM budget exactly 8 banks: gt ring 2 (g chunks, stable 2-allocs/
    iteration phase) + L0 [i,f] 2 + L0-o/c0T staging 1 (tag gos) +
    L1 [i,f] 2 + L1-o/c1T staging 1 (tag l1o). Staging tiles reuse the
    o-gate bank after its sigmoid ACT read; ring slot phase must be
    STABLE across iterations (an odd alloc count per iteration on a
    2-ring couples the chain to the slow consumer every other step).
  - decoder kept in bf16 [batch,h] form (fp8 hd fails the 2e-2 gate;
    the DMA-xbar so.T latency sits on its single-cell recurrence, so
    its tail stays tanh -> h-mul -> 4 PE transposes -> SBUF copy);
    context injected per step via identity matmul; mu/sigma heads are
    DVE dot-products with accumulate.
  - NOTE: scaling the transpose identity to fold in the x16 does NOT
    work - the PE transpose path ignores the identity's values.
"""
import numpy as np
import ml_dtypes

import concourse.bass as bass
import concourse.mybir as mybir
import concourse.tile as tile
from concourse import bacc
from concourse.bass_utils import run_bass_kernel_spmd
from concourse.masks import make_identity

F32 = mybir.dt.float32
BF16 = mybir.dt.bfloat16
FP8 = mybir.dt.float8e4
AF = mybir.ActivationFunctionType
ALU = mybir.AluOpType
DR = mybir.MatmulPerfMode.DoubleRow

B, T_ENC, H_DEC = 1024, 168, 24
ENC_IN, DEC_IN, HID = 32, 16, 512
G = 4 * HID  # 2048
NCORES = 8
BL = B // NCORES  # 128 batch per core
XCHUNK = 28  # encoder-input steps per DMA chunk

WSCALE = 16.0  # fp8 weight pre-scale
HSCALE = 16.0  # fp8 h pre-scale
GSCALE = 1.0 / (WSCALE * HSCALE)  # ACT de-scale on gate reads

# gate reorder: torch order [i, f, g, o] -> [g, i, f, o]
_PERM = np.concatenate([np.arange(1024, 1536), np.arange(0, 512),
                        np.arange(512, 1024), np.arange(1536, 2048)])


def _bf16(x):
    return np.ascontiguousarray(x.astype(ml_dtypes.bfloat16))


def _fp8(x):
    return np.ascontiguousarray(
        np.clip(x, -224.0, 224.0).astype(ml_dtypes.float8_e4m3))


def _f32(x):
    return np.ascontiguousarray(x.astype(np.float32))


def _wT_kxn(W, conv=_bf16, scale=1.0):
    """[4H, D] gate-major weight -> reordered W.T as [128, D//128, 4H]."""
    Wt = W[_PERM].T * scale  # [D, 2048]
    D = Wt.shape[0]
    return conv(Wt.reshape(D // 128, 128, G).transpose(1, 0, 2))


def build_kernel(T=T_ENC, HD=H_DEC):
    nc = bacc.Bacc("TRN2", target_bir_lowering=False, debug=False,
                   num_devices=NCORES)

    def din(name, shape, dt):
        return nc.dram_tensor(name, shape, dt, kind="ExternalInput").ap()

    x_d = din("x", [ENC_IN + 1, T, BL], BF16)        # enc features + ones row
    w0_d = din("w0", [128, G], BF16)  # (W_ih0T + bias row)*256 @ parts 0,64
    wh0_d = din("wh0", [128, 4, G], FP8)              # *16
    wi1_d = din("wi1", [128, 4, G], FP8)              # *16
    wh1_d = din("wh1", [128, 4, G], FP8)              # *16
    wctx_d = din("wctx", [128, 4, G], BF16)
    whd_d = din("whd", [128, 4, G], BF16)
    be_d = din("be", [33, G + 128], BF16)  # row32: bd|ones (decoder)
    # b1*256 by chunk at partitions 0/32/64/96: cols 0:128 ones, 128:640 bias
    b14_d = din("b14", [128, 640], BF16)
    covy_d = din("covy", [128, HD, BL], BF16)  # dec cov+y at parts 0/32/64/96
    wcy_d = din("wcy", [128, G], BF16)         # replicated at parts 0/32/64/96
    # head weights broadcast across partitions + per-partition biases:
    # cols 0:512 W_mu, 512:1024 W_sig, 1024 b_mu, 1025 b_sig
    wms_d = din("wms", [128, 2 * HID + 2], F32)

    mu_d = nc.dram_tensor("mu", [BL, HD], F32, kind="ExternalOutput").ap()
    sg_d = nc.dram_tensor("sg", [BL, HD], F32, kind="ExternalOutput").ap()

    with tile.TileContext(nc) as tc:
        _emit(tc, T, HD, x_d, w0_d, wh0_d, wi1_d, wh1_d, wctx_d, whd_d,
              be_d, b14_d, covy_d, wcy_d, wms_d, mu_d, sg_d)
    nc.compile()
    return nc


def _emit(tc, T, HD, x_d, w0_d, wh0_d, wi1_d, wh1_d, wctx_d, whd_d,
          be_d, b14_d, covy_d, wcy_d, wms_d, mu_d, sg_d):
    nc = tc.nc
    mm = nc.tensor.matmul
    NS = 4  # gate chunks [g, i, f, o]
    CH_ORD = (1, 2, 0, 3)  # emission order [i, f, g, o]: sig([i,f]) first

    with (
        tc.tile_pool(name="const", bufs=1) as cp,
        tc.tile_pool(name="xp", bufs=2) as xp,
        tc.tile_pool(name="sig", bufs=3) as sigp,
        tc.tile_pool(name="small", bufs=3) as smp,
        tc.tile_pool(name="hp", bufs=3) as hp,
        tc.tile_pool(name="htp", bufs=3) as htp,
        tc.tile_pool(name="ht8p", bufs=4) as ht8p,
        # PSUM: 8 banks exactly. tag gt (2 bufs x 1 bank) g chunks;
        # gif (1 x 2 banks) L0 [i,f]; gos (1 x 1) L0 [o] + c0T staging;
        # l1if (1 x 2) L1/dec [i,f]; l1o (1 x 1) L1/dec [o] + c1T staging.
        tc.tile_pool(name="psum", bufs=1, space="PSUM") as pp,
    ):
        # ---- persistent tiles / weight loads ----
        def load(name, dram, shape, dt):
            t = cp.tile(shape, dt, tag=name, name=name)
            nc.sync.dma_start(t[:], dram[:])
            return t

        w0 = load("w0", w0_d, [128, G], BF16)
        wh0 = load("wh0", wh0_d, [128, 4, G], FP8)
        be = load("be", be_d, [33, G + 128], BF16)
        b14 = load("b14", b14_d, [128, 640], BF16)
        wi1 = load("wi1", wi1_d, [128, 4, G], FP8)
        wh1 = load("wh1", wh1_d, [128, 4, G], FP8)

        ident = cp.tile([128, 128], BF16, tag="ident")
        make_identity(nc, ident[:])

        ones32_r = be[32:33, G:G + 128]
        bd_r = be[32:33, 0:G]

        c0 = cp.tile([128, HID], BF16, tag="c0")
        c1 = cp.tile([128, HID], BF16, tag="c1")
        cd = cp.tile([128, HID], BF16, tag="cd")
        mu_b = cp.tile([128, HD], F32, tag="mu_b")
        sp_b = cp.tile([128, HD], F32, tag="sp_b")
        sg_b = cp.tile([128, HD], F32, tag="sg_b")

        def psum_cell(kind):
            """Allocate one cell's gate PSUM tiles -> (dsts, gt, gif, go).

            All cells split as: g 1-bank (tanh), [i,f] 2-bank, [o] 1-bank.
            L0 and L1 use distinct [i,f]/[o] tags so each ring has a
            stable slot per step (no phase alternation); L0's o bank
            doubles as the transpose staging bank (tag gos).
            """
            gt = pp.tile([128, 512], F32, tag="gt", bufs=2, name="gt")
            if kind == "l0":
                gif = pp.tile([128, 1024], F32, tag="gif", bufs=1, name="gif")
                go = pp.tile([128, 512], F32, tag="gos", bufs=1, name="go")
            else:
                gif = pp.tile([128, 1024], F32, tag="l1if", bufs=1, name="gif1")
                go = pp.tile([128, 512], F32, tag="l1o", bufs=1, name="go1")
            dsts = [gt[:], gif[:, 0:512], gif[:, 512:1024], go[:]]
            return dsts, gt, gif, go

        def cell_acts(gt, gif, go, scale):
            """ACT ops for one cell, in chunk-arrival order: the [i,f]
            chunks are emitted first on the PE (see CH_ORD), so
            sigmoid([i,f]) leads the ACT queue and tanh(g) overlaps
            behind it."""
            sif = sigp.tile([128, 1024], BF16, tag="sif")
            nc.scalar.activation(sif[:], gif[:], AF.Sigmoid, scale=scale)
            tg = smp.tile([128, HID], BF16, tag="tg")
            nc.scalar.activation(tg[:], gt[:], AF.Tanh, scale=scale)
            so = sigp.tile([128, HID], BF16, tag="so")
            nc.scalar.activation(so[:], go[:], AF.Sigmoid, scale=scale)
            return tg, sif[:, 0:512], sif[:, 512:1024], so

        def cell_core(tg, si, sf, c, first):
            """DVE c-update (bf16 2x) + tanh(c') on ACT -> tcn tile."""
            if first:
                nc.vector.tensor_mul(c[:], si, tg[:])
            else:
                m1 = smp.tile([128, HID], BF16, tag="m1")
                nc.vector.tensor_mul(m1[:], si, tg[:])
                m2 = smp.tile([128, HID], BF16, tag="m2")
                nc.vector.tensor_mul(m2[:], sf, c[:])
                nc.vector.tensor_add(c[:], m1[:], m2[:])
            tcn = smp.tile([128, HID], BF16, tag="tc")
            nc.scalar.activation(tcn[:], c[:], AF.Tanh)
            return tcn

        def cell_dve(tg, si, sf, so, c, first, h_tag):
            """Full cell tail: h = so * tanh(c') (used by L1/decoder)."""
            tcn = cell_core(tg, si, sf, c, first)
            h = hp.tile([128, HID], BF16, tag=h_tag)
            nc.vector.tensor_mul(h[:], so[:], tcn[:])
            return h

        def pe_transp(h, tag):
            """[128,512] bf16 SBUF -> [128,4,128] bf16 PSUM via 4 PE
            transposes into a staging tile sharing a dead gate bank
            (tag gos = o-chunk bank, tag gif = i/f banks)."""
            ht = pp.tile([128, 4, 128], BF16, tag=tag, bufs=1, name="hT")
            for k in range(4):
                nc.tensor.transpose(ht[:, k, :], h[:, k * 128:(k + 1) * 128],
                                    ident[:])
            return ht

        # ================= encoder =================
        # L1 runs one step behind L0: while L0(t)'s elementwise chain runs
        # on ACT/DVE, the PE stays busy on L1(t-1)'s matmuls.
        h0T8_hist = {}

        x_cur = None
        x_nxt = None

        def load_xchunk(t0):
            """x replicated at partitions 0 and 64 for 2-way row tiling."""
            nxc = min(XCHUNK, T - t0)
            xt = xp.tile([128, XCHUNK, BL], BF16, tag="x")
            nc.sync.dma_start(xt[0:ENC_IN + 1, :nxc, :], x_d[:, t0:t0 + nxc, :])
            nc.sync.dma_start(xt[64:64 + ENC_IN + 1, :nxc, :],
                              x_d[:, t0:t0 + nxc, :])
            return xt

        def l1_bias(d1):
            """K=1 bias matmuls, 4-wide concurrent row groups."""
            for n in CH_ORD:
                bp = 32 * n
                mm(d1[n], b14[bp:bp + 1, 0:128], b14[bp:bp + 1, 128:640],
                   tile_position=(bp, 0), start=True, stop=False)

        def l1_wi1(d1, tp, hp8):
            """wi1 matmuls for layer-1 step tp (all four chunks)."""
            for n in CH_ORD:
                s = slice(n * 512, (n + 1) * 512)
                mm(d1[n], hp8[:, 0:2, :], wi1[:, 0:2, s],
                   perf_mode=DR, start=False, stop=False)
                mm(d1[n], hp8[:, 2:4, :], wi1[:, 2:4, s],
                   perf_mode=DR, start=False, stop=(tp == 0))

        def l1_wh1(d1, h1T8, chunks):
            """wh1 matmuls for the given chunk indices (region stops)."""
            for n in chunks:
                s = slice(n * 512, (n + 1) * 512)
                mm(d1[n], h1T8[:, 0:2, :], wh1[:, 0:2, s],
                   perf_mode=DR, start=False, stop=False)
                mm(d1[n], h1T8[:, 2:4, :], wh1[:, 2:4, s],
                   perf_mode=DR, start=False, stop=True)

        def mchain(tg, si, sf, c, first):
            """DVE c-update (bf16 2x): c' = sf*c + si*tg (in place)."""
            if first:
                nc.vector.tensor_mul(c[:], si, tg[:])
            else:
                m1 = smp.tile([128, HID], BF16, tag="m1")
                nc.vector.tensor_mul(m1[:], si, tg[:])
                m2 = smp.tile([128, HID], BF16, tag="m2")
                nc.vector.tensor_mul(m2[:], sf, c[:])
                nc.vector.tensor_add(c[:], m1[:], m2[:])

        def cell_tail(so, c, ring, so_tag, tc_tag, so_dma=True):
            """Transposed cell tail. In the encoder sigmoid(o).T rides the
            otherwise-idle DMA xbar straight into SBUF (it has ~2us of
            slack before the stt consumes it, saving 4 PE transposes + a
            DVE copy per cell); in the decoder the xbar's ~1.5us latency
            would sit on the recurrence loop, so it stays on the PE.
            c.T takes 4 PE transposes (chain-critical), with tanh
            computed on the transposed PSUM. The h-mul itself is fused
            into the caller's stt/mul, so nothing on the critical path
            waits for a [b,h]-layout h that no one needs."""
            soT_sb = hp.tile([128, 4, 128], BF16, tag=so_tag)
            if so_dma:
                nc.sync.dma_start_transpose(soT_sb[:], so[:])
            else:
                soT = pe_transp(so, ring)
                nc.vector.tensor_copy(soT_sb[:], soT[:])
            cT = pe_transp(c, ring)
            tcnT = smp.tile([128, 4, 128], BF16, tag=tc_tag)
            nc.scalar.activation(tcnT[:], cT[:], AF.Tanh)
            return soT_sb, tcnT

        # cell1 is software-pipelined two-deep: iteration t runs
        #   - L1 matmuls for step t-1 (bias+wi1+wh1),
        #   - cell1(t-1)'s ACTs + c-update (emitted last, execute across
        #     the period boundary),
        #   - cell1(t-2)'s transposed tail -> h1T8(t-2), consumed by
        #     wh1(t-1) in this same iteration.
        # This keeps every emitted op near-ready when its engine reaches
        # it, so the h1 recurrence loop never sets the period.
        h1T8_hist = {}
        pend_so1 = None  # sigmoid(o1) of step t-1 awaiting its transpose

        for t in range(T):
            if t == 0:
                x_cur = load_xchunk(0)
                if T > XCHUNK:
                    x_nxt = load_xchunk(XCHUNK)
            elif t % XCHUNK == 0:
                x_cur = x_nxt
                if t + XCHUNK < T:
                    x_nxt = load_xchunk(t + XCHUNK)
            ti = t % XCHUNK

            # ---- layer 0 step t ----
            # Chunk emission order [i, f, g, o] (CH_ORD): sigmoid([i,f])
            # — the head of the c-update chain — becomes ready after only
            # 4 DR matmuls, with tanh(g) overlapping behind it on ACT.
            # The in-mms pair by PE row group: {i(64)||f(0)}, {g(0)||o(64)};
            # with the L1 bias they also cover the stt0(t-1) chain tail so
            # the PE never idles at the step boundary (idle trips the DVFS
            # throttle: half clock for the next ~3us of matmuls).
            d0, gt0, gif0, go0 = psum_cell("l0")
            hp8 = h0T8_hist.get(t - 1)
            for n in CH_ORD:
                rb = 64 * (n % 2)
                mm(d0[n], x_cur[rb:rb + ENC_IN + 1, ti, :],
                   w0[rb:rb + ENC_IN + 1, n * 512:(n + 1) * 512],
                   tile_position=(rb, 0), start=True, stop=(t == 0))
            if t > 0:
                for n in CH_ORD:
                    s = slice(n * 512, (n + 1) * 512)
                    mm(d0[n], hp8[:, 0:2, :], wh0[:, 0:2, s],
                       perf_mode=DR, start=False, stop=False)
                    mm(d0[n], hp8[:, 2:4, :], wh0[:, 2:4, s],
                       perf_mode=DR, start=False, stop=True)
            # cell0 ACTs (engine waits on the mms via semaphores)
            tg0, si0, sf0, so0 = cell_acts(gt0, gif0, go0, GSCALE)

            # cell1(t-2) transposed tail -> h1T8(t-2) (all inputs are from
            # the previous iteration, so these run without stalling)
            if t >= 2:
                so1T_sb, tcn1T = cell_tail(pend_so1, c1, "l1o", "so1T",
                                           "tc1T")
                h1T8 = ht8p.tile([128, 4, 128], FP8, tag="h1T8")
                nc.vector.scalar_tensor_tensor(
                    h1T8[:], tcn1T[:], HSCALE, so1T_sb[:],
                    op0=ALU.mult, op1=ALU.mult)
                h1T8_hist[t - 2] = h1T8
                h1T8_hist.pop(t - 4, None)

            # L1 psum + ALL FOUR bias mms emitted together here: by the
            # time the PE reaches them every ring is released, so they
            # issue back-to-back into 4 concurrent 32-row groups (one
            # ~310ns slot instead of three staggered ones).
            if t >= 1:
                d1, gt1, gif1, go1 = psum_cell("l1")
                l1_bias(d1)
                l1_wi1(d1, t - 1, h0T8_hist[t - 1])

            # cell0 c-update, then its transposed tail -> h0T8(t)
            mchain(tg0, si0, sf0, c0, t == 0)
            so0T_sb, tcn0T = cell_tail(so0, c0, "gos", "so0T", "tc0T")
            h0T8 = ht8p.tile([128, 4, 128], FP8, tag="h0T8")
            nc.vector.scalar_tensor_tensor(
                h0T8[:], tcn0T[:], HSCALE, so0T_sb[:],
                op0=ALU.mult, op1=ALU.mult)
            h0T8_hist[t] = h0T8
            h0T8_hist.pop(t - 2, None)

            # wh1(t-1) last: h1T8(t-2) lands mid-iteration; these also
            # give the PE tail work past the stt0 chain tail.
            if t >= 2:
                l1_wh1(d1, h1T8_hist[t - 2], CH_ORD)

            # cell1(t-1) ACTs + c-update: execute across the period
            # boundary; the tail runs next iteration.
            if t >= 1:
                tg1, si1, sf1, so1 = cell_acts(gt1, gif1, go1, GSCALE)
                mchain(tg1, si1, sf1, c1, t == 1)
                pend_so1 = so1

        # ---- epilogue: cell1(T-2) tail, then the full final L1 step ----
        if T >= 2:
            so1T_sb, tcn1T = cell_tail(pend_so1, c1, "l1o", "so1T", "tc1T")
            h1T8 = ht8p.tile([128, 4, 128], FP8, tag="h1T8")
            nc.vector.scalar_tensor_tensor(
                h1T8[:], tcn1T[:], HSCALE, so1T_sb[:],
                op0=ALU.mult, op1=ALU.mult)
            h1T8_hist[T - 2] = h1T8
        d1, gt1, gif1, go1 = psum_cell("l1")
        l1_bias(d1)
        l1_wi1(d1, T - 1, h0T8_hist[T - 1])
        if T - 1 > 0:
            l1_wh1(d1, h1T8_hist[T - 2], CH_ORD)
        tg1, si1, sf1, so1 = cell_acts(gt1, gif1, go1, GSCALE)
        mchain(tg1, si1, sf1, c1, T == 1)
        # final h1 in transposed bf16 [128,4,128] for the ctx GEMM
        so1T_sb, tcn1T = cell_tail(so1, c1, "l1o", "so1T", "tc1T")
        h1T = htp.tile([128, 4, 128], BF16, tag="h1T")
        nc.vector.tensor_mul(h1T[:], tcn1T[:], so1T_sb[:])

        # ================= decoder (bf16) =================
        wctx = load("wctx", wctx_d, [128, 4, G], BF16)
        whd = load("whd", whd_d, [128, 4, G], BF16)
        covy = load("covy", covy_d, [128, HD, BL], BF16)
        wcy = load("wcy", wcy_d, [128, G], BF16)
        wms = load("wms", wms_d, [128, 2 * HID + 2], F32)
        # one-time: ctx_pre = context @ W_ctx.T + (b_ihd + b_hhd);
        # bias rides K=1 mms off the be ones row.
        cdst, ctg, ctif, cto = psum_cell("l1")
        for n in range(NS):
            s = slice(n * 512, (n + 1) * 512)
            mm(cdst[n], ones32_r, bd_r[:, s], start=True, stop=False)
        for k in range(4):
            for n in range(NS):
                s = slice(n * 512, (n + 1) * 512)
                mm(cdst[n], h1T[:, k, :], wctx[:, k, s],
                   start=False, stop=(k == 3))
        ctxp = cp.tile([128, G], BF16, tag="ctxp")
        nc.scalar.copy(ctxp[:, 0:512], ctg[:])
        nc.scalar.copy(ctxp[:, 512:1536], ctif[:])
        nc.scalar.copy(ctxp[:, 1536:2048], cto[:])

        hdT = None
        for t in range(HD):
            dd, dgt, dif, dgo = psum_cell("l1")
            for n in CH_ORD:
                s = slice(n * 512, (n + 1) * 512)
                mm(dd[n], ident[:], ctxp[:, s], start=True, stop=False)
            for n in CH_ORD:
                s = slice(n * 512, (n + 1) * 512)
                rb = 32 * n
                mm(dd[n], covy[rb:rb + DEC_IN + 1, t, :],
                   wcy[rb:rb + DEC_IN + 1, s], tile_position=(rb, 0),
                   start=False, stop=(t == 0))
            if t > 0:
                for n in CH_ORD:
                    s = slice(n * 512, (n + 1) * 512)
                    for k in range(4):
                        mm(dd[n], hdT[:, k, :], whd[:, k, s],
                           start=False, stop=(k == 3))
            tgd, sid, sfd, sod = cell_acts(dgt, dif, dgo, 1.0)
            hd = cell_dve(tgd, sid, sfd, sod, cd, t == 0, "hd")
            hdT_ps = pe_transp(hd, "gos")
            hdT = htp.tile([128, 4, 128], BF16, tag="hdT")
            nc.vector.tensor_copy(hdT[:], hdT_ps[:])

            # heads: mu/sigma dot-products on DVE, off the critical path
            hsc = smp.tile([128, HID], F32, tag="hsc")
            nc.vector.scalar_tensor_tensor(
                hsc[:], hd[:], 1.0, wms[:, 0:HID],
                op0=ALU.mult, op1=ALU.mult, accum_out=mu_b[:, t:t + 1])
            hsc2 = smp.tile([128, HID], F32, tag="hsc2")
            nc.vector.scalar_tensor_tensor(
                hsc2[:], hd[:], 1.0, wms[:, HID:2 * HID],
                op0=ALU.mult, op1=ALU.mult, accum_out=sp_b[:, t:t + 1])

        # add head biases; sigma = softplus(x) + 1e-6 via ln(exp(x)+1)
        nc.vector.tensor_scalar_add(mu_b[:], mu_b[:],
                                    wms[:, 2 * HID:2 * HID + 1])
        nc.vector.tensor_scalar_add(sp_b[:], sp_b[:],
                                    wms[:, 2 * HID + 1:2 * HID + 2])
        nc.scalar.activation(sp_b[:], sp_b[:], AF.Exp)
        nc.scalar.activation(sg_b[:], sp_b[:], AF.Ln, bias=1.0)
        nc.vector.tensor_scalar_add(sg_b[:], sg_b[:], 1e-6)
        nc.sync.dma_start(mu_d[:], mu_b[:])
        nc.sync.dma_start(sg_d[:], sg_b[:])


def _make_be(bdv):
    be = np.zeros((33, G + 128), np.float32)
    be[32, :G] = bdv
    be[32, G:] = 1.0
    return _bf16(be)


def _make_b14(b1):
    """b1*256 chunks at partitions 0/32/64/96 for 4-wide K=1 row tiling."""
    b = np.zeros((128, 640), np.float32)
    for i in range(4):
        b[32 * i, 0:128] = 1.0
        b[32 * i, 128:640] = b1[i * 512:(i + 1) * 512] / GSCALE
    return _bf16(b)


def _make_wms(W_mu, W_sig, b_mu, b_sig):
    w = np.zeros((128, 2 * HID + 2), np.float32)
    w[:, 0:HID] = W_mu[0][None, :]
    w[:, HID:2 * HID] = W_sig[0][None, :]
    w[:, 2 * HID] = b_mu[0]
    w[:, 2 * HID + 1] = b_sig[0]
    return _f32(w)


def prep_inputs(inputs, T=T_ENC, HD=H_DEC):
    """Full-batch inputs -> list of per-core input maps (host layout prep)."""
    enc = _f32(np.asarray(inputs["enc_inp"]))[:, :T]
    dec = _f32(np.asarray(inputs["dec_inp"]))[:, :HD]
    tgt = _f32(np.asarray(inputs["tgt"]))[:, :HD]

    W_ih0, W_hh0 = np.asarray(inputs["W_ih0"]), np.asarray(inputs["W_hh0"])
    W_ih1, W_hh1 = np.asarray(inputs["W_ih1"]), np.asarray(inputs["W_hh1"])
    W_ihd, W_hhd = np.asarray(inputs["W_ihd"]), np.asarray(inputs["W_hhd"])
    b0 = _f32(np.asarray(inputs["b_ih0"]) + np.asarray(inputs["b_hh0"]))[_PERM]
    b1 = _f32(np.asarray(inputs["b_ih1"]) + np.asarray(inputs["b_hh1"]))[_PERM]
    bdv = _f32(np.asarray(inputs["b_ihd"]) + np.asarray(inputs["b_hhd"]))[_PERM]
    W_mu, b_mu = np.asarray(inputs["W_mu"]), np.asarray(inputs["b_mu"])
    W_sig, b_sig = np.asarray(inputs["W_sig"]), np.asarray(inputs["b_sig"])

    # x-side weights *256 in bf16 (exact power-of-two scale); gate reads
    # apply scale=1/256. b0 rides the ones-row of x. w0 replicated at
    # partition 64 for the 2-wide 64-row input matmuls.
    w0 = np.concatenate([W_ih0[_PERM].T, b0[None, :]], 0) / GSCALE  # [33,2048]
    w02 = np.zeros((128, G), np.float32)
    w02[0:ENC_IN + 1] = w0
    w02[64:64 + ENC_IN + 1] = w0
    wcy1 = np.concatenate(
        [W_ihd[_PERM][:, :DEC_IN].T, W_ihd[_PERM][:, DEC_IN + HID:].T], 0)
    wcy4 = np.zeros((128, G), np.float32)
    for i in range(4):
        wcy4[32 * i:32 * i + DEC_IN + 1] = wcy1
    shared = {
        "w0": _bf16(w02),
        "wh0": _wT_kxn(W_hh0, conv=_fp8, scale=WSCALE),
        "wi1": _wT_kxn(W_ih1, conv=_fp8, scale=WSCALE),
        "wh1": _wT_kxn(W_hh1, conv=_fp8, scale=WSCALE),
        "wctx": _wT_kxn(W_ihd[:, DEC_IN:DEC_IN + HID]),
        "whd": _wT_kxn(W_hhd),
        "be": _make_be(bdv),
        "b14": _make_b14(b1),
        "wcy": _bf16(wcy4),
        "wms": _make_wms(W_mu, W_sig, b_mu, b_sig),
    }

    in_maps = []
    for c in range(NCORES):
        sl = slice(c * BL, (c + 1) * BL)
        xe = np.ones((ENC_IN + 1, T, BL), np.float32)
        xe[:ENC_IN] = enc[sl].transpose(2, 1, 0)
        cy1 = np.zeros((DEC_IN + 1, HD, BL), np.float32)
        cy1[:DEC_IN] = dec[sl].transpose(2, 1, 0)
        cy1[DEC_IN, 1:] = tgt[sl, :HD - 1].T
        cy = np.zeros((128, HD, BL), np.float32)
        for i in range(4):
            cy[32 * i:32 * i + DEC_IN + 1] = cy1
        m = dict(shared)
        m["x"] = _bf16(xe)
        m["covy"] = _bf16(cy)
        in_maps.append(m)
    return in_maps


_NC_CACHE = {}


def _get_nc(T=T_ENC, HD=H_DEC):
    key = (T, HD)
    if key not in _NC_CACHE:
        _NC_CACHE[key] = build_kernel(T, HD)
    return _NC_CACHE[key]


def run(inputs, T=T_ENC, HD=H_DEC, **kw):
    nc = _get_nc(T, HD)
    in_maps = prep_inputs(inputs, T, HD)
    res = run_bass_kernel_spmd(nc, in_maps, core_ids=list(range(NCORES)), **kw)
    mu = np.concatenate([res.results[c]["mu"] for c in range(NCORES)], 0)
    sg = np.concatenate([res.results[c]["sg"] for c in range(NCORES)], 0)
    return (mu, sg), res


def kernel(**inputs):
    (mu, sg), _ = run(inputs)
    return mu, sg
